# revision 44
# baseline (speedup 1.0000x reference)
"""Trainium2 Bass kernel for a PointNet++-style feature-propagation decoder
(4 stages of kNN(k=3) inverse-distance-weighted feature interpolation).

Sharding: batch b = core//2 (data parallel over B=4), and the finest stage's
8192 query points split in half across each core pair (point parallel along N
per the sharding hint). Stages 0-2 are duplicated within a pair (cheap);
stage 3 dominates and is n-split. Output rows 0:64 are the x0 passthrough,
assembled on the host.

Per-core device pipeline per stage (v3 — fp16 datapath):
  - negated squared distances via one K=13 PE matmul (fp32r fast-fp32 mode)
    per 128-query tile. fp32r keeps only 11 mantissa bits, which would flip
    kNN selections vs the f32 reference, so the matmul uses compensated
    hi/lo-split operands (x*y ~ xh*yh + xl*yh + xh*yl, built on the HOST
    with matching 11-bit rounding):
      A rows = [ah, al, ah, nh, nl, -1, -1]   (n = -|a|^2)
      B rows = [bh, bh, bl, 1, 1, mh, ml]     (b = 2*coarse, m = +|b|^2)
      A.B = -dist to full fp32 accuracy
  - dist copied PSUM->SBUF as fp16 (ACT), top-3 via DVE max8 + max_index
  - inverse-distance weights on DVE (f32), d clamped to >=EPS (guards the
    residual negative-distance cancellation)
  - feature rows gathered fp16 via SWDGE indirect DMA (row per partition)
  - weighted 3-way combine on the PE: psum += diag(w_k) @ g_k, where
    diag(w_k) is built as identity * per-partition scale (ACT/DVE)
  - psum -> SBUF fp16 copies split across ACT/DVE; stage output rows are
    DMA'd into the next stage's fp16 DRAM source table; the final stage is
    written row-major [n, D] fp16 and transposed/cast on the host during
    unshard.
"""

import numpy as np

P = 128
KNN = 3
EPS = 1e-8

B = 4
NS = [8192, 2048, 512, 128, 32]  # points per level, finest -> coarsest
CS = [64, 128, 256, 512, 1024]   # feature channels per level

_CACHED = {"nc": None, "key": None}


def _stage_dims(ns, cs, n_half):
    d2 = [cs[4]]
    for s in range(1, 4):
        d2.append(cs[4 - s] + d2[s - 1])
    stages = []
    for s in range(4):
        nf = ns[3 - s] if s < 3 else n_half
        stages.append(dict(nf=nf, S=ns[4 - s], d2=d2[s],
                           cx=(cs[3 - s] if s < 3 else None)))
    return stages


def _build_program(ns, cs, n_half, split_waits=True):
    """Trace the per-core Bass program. ns/cs as in reference (finest first).
    n_half: number of finest-level query points this core handles."""
    import contextlib

    import concourse.bass as bass
    import concourse.mybir as mybir
    import concourse.tile as tile
    from concourse.bass import IndirectOffsetOnAxis
    from concourse.masks import make_identity

    _patch_tile_drain()

    f32 = mybir.dt.float32
    f32r = mybir.dt.float32r
    f16 = mybir.dt.float16
    u32 = mybir.dt.uint32
    Alu = mybir.AluOpType
    Axis = mybir.AxisListType
    Act = mybir.ActivationFunctionType

    stages = _stage_dims(ns, cs, n_half)

    nc = bass.Bass("TRN2")

    # ---- external inputs (per core) ----
    # compensated fp32r distance operands, assembled on the host
    amat = {}
    bmat = {}
    for s, st in enumerate(stages):
        amat[s] = nc.dram_tensor(f"a13_{s}", [13, st["nf"]], f32r,
                                 kind="ExternalInput")
        bmat[s] = nc.dram_tensor(f"b13_{s}", [13, st["S"]], f32r,
                                 kind="ExternalInput")
    feat = {}
    for i in range(1, 5):
        feat[i] = nc.dram_tensor(f"f{i}", [cs[i], ns[i]], f32, kind="ExternalInput")

    # ---- external output: interp part of final stage, ROW layout, fp16 ----
    oi = nc.dram_tensor("oi", [n_half, stages[3]["d2"]], f16,
                        kind="ExternalOutput")

    # ---- internal DRAM gather tables (fp16, cascade-collapsed) ----
    # Stage-2 rows r hold [x2^T[r] | W1[r]]: the stage-1 interp is never
    # materialized; its 1536-wide value is (W1 @ T1)[r] with T1 SBUF-resident.
    # Stage-3 rows hold [x1^T | x2part | M] where [x2part | M] is stage-2's
    # collapsed output; the full 1920-wide output row is
    # [x1part | x2part | N @ T1] with N the gathered-combined M rows.
    tgat2 = nc.dram_tensor("tgat2", [stages[2]["S"], cs[2] + ns[3]], f16)
    tgat3 = nc.dram_tensor("tgat3", [stages[3]["S"], cs[1] + cs[2] + ns[3]],
                           f16)

    with tile.TileContext(nc) as tc, contextlib.ExitStack() as ctx:
        cpool = ctx.enter_context(tc.tile_pool(name="const", bufs=1))
        inpool = ctx.enter_context(tc.tile_pool(name="in", bufs=1))
        abpool = ctx.enter_context(tc.tile_pool(name="ab", bufs=1))
        ndpool = ctx.enter_context(tc.tile_pool(name="nd", bufs=3))
        smpool = ctx.enter_context(tc.tile_pool(name="sm", bufs=2))
        gpool = ctx.enter_context(tc.tile_pool(name="g", bufs=3))
        rpool = ctx.enter_context(tc.tile_pool(name="r", bufs=3))
        dgpool = ctx.enter_context(tc.tile_pool(name="dg", bufs=4))
        xtp = ctx.enter_context(tc.tile_pool(name="xtp", bufs=2))
        ps_nd = ctx.enter_context(tc.tile_pool(name="ps_nd", bufs=2, space="PSUM"))
        ps_cb = ctx.enter_context(tc.tile_pool(name="ps_cb", bufs=4, space="PSUM"))

        ident = cpool.tile([P, P], f32, tag="ident")
        make_identity(nc, ident[:])
        ident16 = cpool.tile([P, P], f16, tag="ident16")
        nc.vector.tensor_copy(ident16[:], ident[:])
        # iota row 0..127 replicated on every partition (f32, exact ints)
        iota_i = cpool.tile([P, P], mybir.dt.int32, tag="iota_i")
        nc.gpsimd.iota(iota_i[:], pattern=[[1, P]], base=0,
                       channel_multiplier=0)
        iota_f = cpool.tile([P, P], f32, tag="iota_f")
        nc.vector.tensor_copy(iota_f[:], iota_i[:])

        # SBUF-resident source tables for the two small stages
        t0sb = cpool.tile([stages[0]["S"], stages[0]["d2"]], f16, tag="t0sb")
        t1sb = cpool.tile([stages[1]["S"], stages[1]["d2"]], f16, tag="t1sb")

        # ---------- build x^T parts of the source tables ----------
        def xt_transpose(fi, nrows, ncols, sink):
            """sink(t, c0, psum_ap, rows, cw): receives x^T [rows, cw] chunks
            ([nrows points, ncols feats] overall) from PE transposes."""
            cchunks = (ncols + P - 1) // P
            rchunks = (nrows + P - 1) // P
            fsb = inpool.tile([min(ncols, P), cchunks * nrows], f32,
                              tag="fsb")
            src = feat[fi][:, :]
            if ncols > P:
                nc.sync.dma_start(
                    fsb[:].rearrange("p (cc n) -> p cc n", cc=cchunks),
                    src.rearrange("(cc p) n -> p cc n", p=P))
            else:
                nc.sync.dma_start(fsb[:ncols, :nrows], src)
            for t in range(rchunks):
                rows = min(P, nrows - t * P)
                for cc in range(cchunks):
                    c0 = cc * P
                    cw = min(P, ncols - c0)
                    pst = ps_cb.tile([P, 512], f32, tag="cmb")
                    nc.tensor.transpose(
                        pst[:rows, :cw],
                        fsb[:cw, cc * nrows + t * P: cc * nrows + t * P + rows],
                        ident[:])
                    sink(t, c0, pst, rows, cw)

        def _psum_copy(dst_ap, src_ap, parity):
            # split PSUM->SBUF copies across ACT/DVE by parity
            if parity % 2 == 0:
                nc.scalar.copy(dst_ap, src_ap)
            else:
                nc.vector.tensor_copy(dst_ap, src_ap)

        def xt_into_dram(fi, table, nrows, ncols):
            state = {}

            def sink(t, c0, pst, rows, cw):
                if t not in state:
                    xtt_new = xtp.tile([P, ((ncols + P - 1) // P) * P], f16,
                                       tag="xtt")
                    state[t] = xtt_new
                xtt = state[t]
                nc.scalar.copy(xtt[:rows, c0:c0 + cw], pst[:rows, :cw])
                if c0 + cw >= ncols:
                    nc.sync.dma_start(table[t * P:t * P + rows, 0:ncols],
                                      xtt[:rows, :ncols])
            xt_transpose(fi, nrows, ncols, sink)

        def xt_into_sbuf(fi, tile_dst, nrows, ncols):
            def sink(t, c0, pst, rows, cw):
                nc.scalar.copy(tile_dst[t * P:t * P + rows, c0:c0 + cw],
                               pst[:rows, :cw])
            xt_transpose(fi, nrows, ncols, sink)

        xt_into_sbuf(4, t0sb, stages[0]["S"], cs[4])       # x4^T -> T0 (SBUF)
        xt_into_sbuf(3, t1sb, stages[1]["S"], cs[3])       # x3^T -> T1 (SBUF)
        xt_into_dram(2, tgat2, stages[2]["S"], cs[2])      # x2^T -> tgat2
        xt_into_dram(1, tgat3, stages[3]["S"], cs[1])      # x1^T -> tgat3

        # ---------- stages ----------
        for s, st in enumerate(stages):
            nf, S, D2 = st["nf"], st["S"], st["d2"]
            T = nf // P
            a13 = abpool.tile([13, st["nf"]], f32r, tag=f"a13_{s}")
            nc.sync.dma_start(a13[:], amat[s][:, :])
            b13 = abpool.tile([13, st["S"]], f32r, tag=f"b13_{s}")
            nc.sync.dma_start(b13[:], bmat[s][:, :])

            # ---- block-pipelined dist+top3 / weights / gather+combine ----
            maxb = smpool.tile([P, T * 8], f16, tag=f"maxb{s}")
            idxb = smpool.tile([P, T * 8], u32, tag=f"idxb{s}")
            dbuf = smpool.tile([P, T * KNN], f32, tag=f"dbuf{s}")
            wraw = smpool.tile([P, T * KNN], f32, tag=f"wraw{s}")
            wsum = smpool.tile([P, T], f32, tag=f"wsum{s}")
            wnrm = smpool.tile([P, T], f32, tag=f"wnrm{s}")
            wgt = smpool.tile([P, T * KNN], f32, tag=f"wgt{s}")
            nchunk = (S + 511) // 512
            # output col chunking for combine psum (one 512-f32 bank each)
            ocw = []
            o0 = 0
            while o0 < D2:
                w_ = min(512, D2 - o0)
                ocw.append((o0, w_))
                o0 += w_
            BLK = 2

            def emit_topk(b0, bn):
              for t in range(b0, b0 + bn):
                nd_sb = ndpool.tile([P, S], f16, tag="nd_sb")
                pnd = ps_nd.tile([P, min(S, 1024)], f32, tag="pnd")
                for c in range(nchunk):
                    w = min(512, S - c * 512)
                    nc.tensor.matmul(
                        pnd[:, (c % 2) * 512:(c % 2) * 512 + w],
                        a13[:, t * P:(t + 1) * P],
                        b13[:, c * 512:c * 512 + w],
                        start=True, stop=True)
                    if c % 2 == 1 or c == nchunk - 1:
                        lo = (c // 2) * 1024
                        w2 = min(1024, S - lo)
                        nc.scalar.copy(nd_sb[:, lo:lo + w2], pnd[:, :w2])
                        if c != nchunk - 1:
                            pnd = ps_nd.tile([P, min(S, 1024)], f32, tag="pnd")
                nc.vector.max(maxb[:, t * 8:(t + 1) * 8], nd_sb[:])
                nc.vector.max_index(idxb[:, t * 8:(t + 1) * 8],
                                    maxb[:, t * 8:(t + 1) * 8], nd_sb[:])

              # weights for this block
              top3 = maxb[:, b0 * 8:(b0 + bn) * 8].rearrange(
                  "p (t e) -> p t e", e=8)[:, :, 0:KNN]
              d3 = dbuf[:, b0 * KNN:(b0 + bn) * KNN]
              w3 = wraw[:, b0 * KNN:(b0 + bn) * KNN]
              g3 = wgt[:, b0 * KNN:(b0 + bn) * KNN]
              # d = max(-top3, EPS): the max-clamp (vs reference's d+EPS)
              # guards against tiny negative distances from residual rounding
              nc.vector.tensor_scalar(d3.rearrange("p (t e) -> p t e", e=KNN),
                                      top3, -1.0, EPS, op0=Alu.mult, op1=Alu.max)
              nc.vector.reciprocal(w3, d3)
              nc.vector.tensor_reduce(
                  wsum[:, b0:b0 + bn], w3.rearrange("p (t e) -> p t e", e=KNN),
                  axis=Axis.X, op=Alu.add)
              nc.vector.reciprocal(wnrm[:, b0:b0 + bn], wsum[:, b0:b0 + bn])
              nc.vector.tensor_tensor(
                  g3.rearrange("p (t e) -> p t e", e=KNN),
                  w3.rearrange("p (t e) -> p t e", e=KNN),
                  wnrm[:, b0:b0 + bn].rearrange(
                      "p (t o) -> p t o", o=1).to_broadcast([P, bn, KNN]),
                  op=Alu.mult)

            def emit_gc(b0, bn):
              if s < 2:
                # ---- selection-matmul form (SBUF-resident table) ----
                # W[q, s2] = sum_k (iota==idx_k[q]) * w_k[q]; out = W @ tbl
                idxf = smpool.tile([P, T * KNN], f32, tag=f"idxf{s}")
                nc.vector.tensor_copy(
                    idxf[:, b0 * KNN:(b0 + bn) * KNN].rearrange(
                        "p (t e) -> p t e", e=KNN),
                    idxb[:, b0 * 8:(b0 + bn) * 8].rearrange(
                        "p (t e) -> p t e", e=8)[:, :, 0:KNN])
                for t in range(b0, b0 + bn):
                    wq = rpool.tile([P, S], f32, tag=f"wq{s}")
                    for k in range(KNN):
                        if k == 0:
                            dst = wq
                        else:
                            dst = rpool.tile([P, S], f32, tag=f"mk{s}")
                        nc.vector.scalar_tensor_tensor(
                            dst[:].rearrange("p (o s2) -> p o s2", o=1),
                            iota_f[:, :S].rearrange("p (o s2) -> p o s2", o=1),
                            idxf[:, t * KNN + k:t * KNN + k + 1],
                            wgt[:, t * KNN + k:t * KNN + k + 1].rearrange(
                                "p (t2 o) -> p t2 o", o=1).to_broadcast(
                                    [P, 1, S]),
                            op0=Alu.is_equal, op1=Alu.mult)
                        if k > 0:
                            nc.vector.tensor_tensor(wq[:], wq[:], dst[:],
                                                    op=Alu.add)
                    if s == 1:
                        # stage 1 collapses: only its selection weights W1
                        # are needed downstream (tgat2 cols 256:384)
                        wq16 = rpool.tile([P, P], f16, tag="wq16")
                        nc.scalar.copy(wq16[:], wq[:])
                        nc.sync.dma_start(
                            tgat2[t * P:(t + 1) * P, cs[2]:cs[2] + ns[3]],
                            wq16[:])
                        continue
                    # stage 0: materialize interp into T1 (SBUF)
                    ptw = ps_cb.tile([P, 512], f32, tag="cmb")
                    nc.tensor.transpose(ptw[:S, :P], wq[:], ident[:])
                    wt16 = rpool.tile([P, P], f16, tag="wt16")
                    nc.scalar.copy(wt16[:S, :], ptw[:S, :P])
                    for ci, (c0, cw) in enumerate(ocw):
                        pcb = ps_cb.tile([P, 512], f32, tag="cmb")
                        nc.tensor.matmul(pcb[:, :cw], wt16[:S, :],
                                         t0sb[:S, c0:c0 + cw],
                                         start=True, stop=True)
                        dst_ap = t1sb[:, cs[3] + c0:cs[3] + c0 + cw]
                        if ci % 2 == 0:
                            nc.scalar.copy(dst_ap, pcb[:, :cw])
                        else:
                            nc.vector.tensor_copy(dst_ap, pcb[:, :cw])
                return

              # ---- gather + PE diag-combine (stages 2/3, collapsed) ----
              gsrc = tgat2 if s == 2 else tgat3
              GW = (cs[2] + ns[3]) if s == 2 else (cs[1] + cs[2] + ns[3])
              for t in range(b0, b0 + bn):
                gt = gpool.tile([P, KNN * GW], f16, tag=f"gt{s}")
                for k in range(KNN):
                    nc.gpsimd.indirect_dma_start(
                        out=gt[:, k * GW:(k + 1) * GW],
                        out_offset=None,
                        in_=gsrc[:, :],
                        in_offset=IndirectOffsetOnAxis(
                            ap=idxb[:, t * 8 + k:t * 8 + k + 1], axis=0))

                # diag(w_k): identity * per-partition scale (ACT)
                diags = []
                for k in range(KNN):
                    dg = dgpool.tile([P, P], f16, tag=f"dg{k}")
                    nc.scalar.activation(dg[:], ident16[:], Act.Identity,
                                         scale=wgt[:, t * 3 + k:t * 3 + k + 1])
                    diags.append(dg)

                pcb = ps_cb.tile([P, 512], f32, tag="cmb")
                for k in range(KNN):
                    nc.tensor.matmul(
                        pcb[:, :GW],
                        diags[k][:],
                        gt[:, k * GW:k * GW + GW],
                        start=(k == 0), stop=(k == KNN - 1))

                if s == 2:
                    # out rows: [x2part | M] -> tgat3 cols 128:512
                    o16 = rpool.tile([P, cs[2] + ns[3]], f16, tag="o16")
                    nc.scalar.copy(o16[:], pcb[:, :GW])
                    nc.sync.dma_start(
                        tgat3[t * P:(t + 1) * P, cs[1]:cs[1] + GW], o16[:])
                else:
                    # out rows: [x1part | x2part | N @ T1]
                    out_sb = rpool.tile([P, D2], f16, tag="out_sb")
                    nc.scalar.copy(out_sb[:, 0:cs[1] + cs[2]],
                                   pcb[:, 0:cs[1] + cs[2]])
                    n16 = rpool.tile([P, P], f32, tag="n16")
                    nc.scalar.copy(n16[:], pcb[:, cs[1] + cs[2]:GW])
                    # matmul wants N^T as the stationary operand
                    ptn = ps_cb.tile([P, 512], f32, tag="cmb")
                    nc.tensor.transpose(ptn[:, :P], n16[:], ident[:])
                    nt16 = rpool.tile([P, P], f16, tag="nt16")
                    nc.scalar.copy(nt16[:], ptn[:, :P])
                    w1d2 = cs[3] + cs[4]  # 1536 = T1 width
                    for ci in range(w1d2 // 512):
                        pc2 = ps_cb.tile([P, 512], f32, tag="cmb")
                        nc.tensor.matmul(pc2[:], nt16[:],
                                         t1sb[:, ci * 512:(ci + 1) * 512],
                                         start=True, stop=True)
                        dst_ap = out_sb[:, cs[1] + cs[2] + ci * 512:
                                        cs[1] + cs[2] + (ci + 1) * 512]
                        if ci == 1:
                            nc.vector.tensor_copy(dst_ap, pc2[:])
                        else:
                            nc.scalar.copy(dst_ap, pc2[:])
                    nc.sync.dma_start(oi[t * P:(t + 1) * P, :], out_sb[:])

            # one-block software-pipeline skew: block i's topk overlaps
            # block i-1's gather/combine (keeps DVE and Pool/PE/ACT co-busy)
            blocks = [(b0, min(BLK, T - b0)) for b0 in range(0, T, BLK)]
            prev = None
            for blk in blocks:
                emit_topk(*blk)
                if prev is not None:
                    emit_gc(*prev)
                prev = blk
            emit_gc(*prev)
    if split_waits:
        _split_multi_waits(nc)
    return nc


def _split_multi_waits(nc):
    """This walrus build rejects instructions carrying more than one sync
    wait. Hoist extra waits into same-engine NoOps inserted just before."""
    import concourse.mybir as mybir

    n = 0
    for f in nc.m.functions:
        for bb in f.blocks:
            il = bb.instructions
            i = 0
            while i < len(il):
                inst = il[i]
                si = getattr(inst, "sync_info", None)
                ow = list(si.on_wait) if si is not None else []
                if len(ow) > 1:
                    for w in ow[:-1]:
                        nop = mybir.InstNoOp(name=f"W{n}-{inst.name}",
                                             ins=[], outs=[])
                        n += 1
                        nop.engine = inst.engine
                        nop.sync_info = mybir.SyncInfo(on_update=[],
                                                       on_wait=[w])
                        il.insert(i, nop)
                        i += 1
                    inst.sync_info = mybir.SyncInfo(
                        on_update=list(si.on_update), on_wait=[ow[-1]])
                i += 1


def _patch_tile_drain():
    """This walrus build rejects >1 sync-wait on the kernel-tail Drain; spread
    the waits across single-wait SP nops instead."""
    import concourse.mybir as mybir
    import concourse.tile as tile
    from concourse.vector_clock import ScopedClock

    if getattr(tile.TileContext, "_drain_patched", False):
        return

    def _patched(self, tick_clock, wait_clock):
        nc = self.nc
        probe = nc.sync.nop()
        wait_clock.add_sem_waits(probe.ins,
                                 ScopedClock({None: tick_clock.global_clock}))
        si = probe.ins.sync_info
        ow = list(si.on_wait) if si is not None else []
        if len(ow) > 1:
            for w in ow[1:]:
                n2 = nc.sync.nop()
                n2.ins.sync_info = mybir.SyncInfo(on_update=[], on_wait=[w])
            probe.ins.sync_info = mybir.SyncInfo(on_update=list(si.on_update),
                                                 on_wait=[ow[0]])
        nc.sync.drain()
        nc.all_engine_barrier()
        assert self.sems is not None
        popped = nc._tile_sem_poison_stack.pop()
        assert popped is self._sem_poison
        nc.clear_and_free_semaphores(list(self.sems.allocated().values()))
        nc.all_engine_barrier()

    tile.TileContext._drain_and_barrier = _patched
    tile.TileContext._drain_patched = True


def _get_program(ns, cs, n_half):
    key = (tuple(ns), tuple(cs), n_half)
    if _CACHED["key"] != key:
        _CACHED["nc"] = _build_program(ns, cs, n_half)
        _CACHED["key"] = key
    return _CACHED["nc"]


def _r11(x):
    """Round float32 to 11 mantissa bits (matches TRN2 fp32r), half-up."""
    xb = np.ascontiguousarray(x, dtype=np.float32).view(np.uint32)
    rounded = ((xb.astype(np.uint64) + (1 << 11)) >> 12 << 12).astype(np.uint32)
    return rounded.view(np.float32)


def _hilo_operands(xf, xc):
    """Host-side compensated fp32r distance operands.
    xf: [nf, 3] fine points, xc: [S, 3] coarse points ->
    A13 [13, nf], B13 [13, S] float32 with A13.T @ B13 = -dist."""
    nf, S = xf.shape[0], xc.shape[0]
    a = np.ascontiguousarray(xf.T, dtype=np.float32)
    b2 = np.ascontiguousarray(2.0 * xc.T, dtype=np.float32)
    na = -np.sum(xf.astype(np.float32) ** 2, axis=1)[None, :]
    nb = np.sum(xc.astype(np.float32) ** 2, axis=1)[None, :]
    ah = _r11(a)
    al = a - ah
    nh = _r11(na)
    nl = na - nh
    bh = _r11(b2)
    bl = b2 - bh
    mh = _r11(nb)
    ml = nb - mh
    A13 = np.concatenate(
        [ah, al, ah, nh, nl,
         -np.ones((2, nf), np.float32)], axis=0).astype(np.float32)
    B13 = np.concatenate(
        [bh, bh, bl, np.ones((2, S), np.float32), mh, ml],
        axis=0).astype(np.float32)
    return np.ascontiguousarray(A13), np.ascontiguousarray(B13)


def make_core_inputs(inputs, ns, cs, n_half, core):
    """Slice/transform full inputs for one core (b = core//2, half = core%2)."""
    b, h = core // 2, core % 2
    xyz = [np.asarray(inputs[f"xyz{i}"])[b] for i in range(5)]
    xyz0h = xyz[0][h * n_half:(h + 1) * n_half]
    stages = _stage_dims(ns, cs, n_half)
    d = {}
    for s in range(4):
        fine = xyz[3 - s] if s < 3 else xyz0h
        coarse = xyz[4 - s]
        A13, B13 = _hilo_operands(fine, coarse)
        d[f"a13_{s}"] = A13
        d[f"b13_{s}"] = B13
    for i in range(1, 5):
        d[f"f{i}"] = np.ascontiguousarray(np.asarray(inputs[f"x{i}"])[b])
    return d


def kernel(**inputs):
    from concourse.bass_utils import run_bass_kernel_spmd

    ns, cs = NS, CS
    n_half = ns[0] // 2
    nc = _get_program(ns, cs, n_half)

    in_maps = [make_core_inputs(inputs, ns, cs, n_half, c) for c in range(8)]
    res = run_bass_kernel_spmd(nc, in_maps, core_ids=list(range(8)))

    dout = sum(cs)
    out = np.empty((B, dout, ns[0]), np.float32)
    out[:, :cs[0], :] = np.asarray(inputs["x0"])
    for c in range(8):
        b, h = c // 2, c % 2
        out[b, cs[0]:, h * n_half:(h + 1) * n_half] = \
            np.asarray(res.results[c]["oi"]).astype(np.float32).T
    return out
